# revision 22
# baseline (speedup 1.0000x reference)
"""Trainium2 Bass kernel for nn_Catting_75058848465342.

Reference:  out = swapaxes(x[:, :, :64, :], -1, -2).reshape(B, C, N*S)
with x: [B=16, C=64, S=64, N=512] f32 — a pure data-movement problem
(transpose of the last two axes; the slice is the full dim).

Sharding: data-parallel over B across 8 NeuronCores (2 batches per core).

Per-core layout (memory-bound, target ~HBM roofline):
  * the 128 [S=64, N=512] matrices are processed in stacked PAIRS: the load
    DMA places a pair as an SBUF tile [128 part = (m, s), 512] so loads are
    2KB-contiguous per partition and 2MB per dma_start (8 pairs batched).
  * 4 TensorE transpose-mode matmuls per pair, with a stride-4 column access
    pattern on the stationary operand (columns n = 4p+t feed PSUM partition
    p), so PSUM partition p holds output columns n = 4p..4p+3 -> the store
    sees 1KB-contiguous HBM runs.
  * DVE copies PSUM->SBUF, reordering (t, m, s) -> (m, t, s).
  * store DMAs are 2MB per instruction (16 matrices), issued on the second
    HWDGE ring (scalar/ACT) so loads and stores use separate rings.

Measured on 8 axon trn2 cores: ~108 us/iteration per core for 32MB of HBM
traffic (~310 GB/s/core mixed R+W); DMA-bound — PE/DVE work is fully hidden
(a DMA-only variant measures the same time).
"""
import sys

try:
    import concourse  # noqa: F401
except ImportError:
    sys.path.insert(0, "/opt/trn_rl_repo")

import numpy as np
from contextlib import ExitStack

from concourse import bacc, bass_utils, tile, masks
import concourse.mybir as mybir

F32 = mybir.dt.float32

N_CORES = 8
B, C, S, N = 16, 64, 64, 512
B_PER = B // N_CORES          # 2 batches per core
MATS = B_PER * C              # 128 [64,512] matrices per core
PAIRS = MATS // 2             # 64 stacked pairs
SUPER = 8                     # pairs per DMA super-tile (8 pairs = 16 mats = 2MB)
N_SUPER = PAIRS // SUPER      # 8 super-iterations
BUFS = 3

_CACHE = {}


def _build(repeat: int = 1):
    n_super = N_SUPER
    nc = bacc.Bacc("TRN2", target_bir_lowering=False, debug=False, num_devices=N_CORES)
    # x per core: [64 pairs, 128 rows=(m,s), 512 cols=n]  (same bytes as
    # [2, 64, 64, 512] row-major)
    x = nc.dram_tensor("x", [PAIRS, 128, N], F32, kind="ExternalInput").ap()
    # out per core: [sup, mat16, p, (t,s)] — flat bytes equal out[mat, n*64+s]
    out = nc.dram_tensor("out", [n_super, 2 * SUPER, 128, 256], F32,
                         kind="ExternalOutput").ap()

    with ExitStack() as ctx:
        tc = ctx.enter_context(tile.TileContext(nc))
        const_pool = ctx.enter_context(tc.tile_pool(name="const", bufs=1))
        in_pool = ctx.enter_context(tc.tile_pool(name="in", bufs=BUFS))
        out_pool = ctx.enter_context(tc.tile_pool(name="out", bufs=BUFS))
        psum_pool = ctx.enter_context(tc.tile_pool(name="psum", bufs=8, space="PSUM"))

        ident = const_pool.tile([128, 128], F32)
        masks.make_identity(nc, ident[:])

        def body():
            for sup in range(n_super):
                # load 8 pairs = 2MB: dram (pair', part, n) -> (part, pair', n)
                tin = in_pool.tile([128, SUPER, 128, 4], F32)  # (pair', n_hi, t)
                nc.sync.dma_start(
                    tin[:], x[sup * SUPER:(sup + 1) * SUPER].transpose([1, 0, 2]))
                tout = out_pool.tile([128, 2 * SUPER, 4, 64], F32)  # ((pair',m), t, s)
                for q in range(SUPER):
                    psum_t = psum_pool.tile([128, 4, 2, 64], F32)  # one bank: (t, m, s)
                    for t in range(4):
                        # stationary = tin[:, q, :, t]: [128 part, 128 cols stride 4]
                        # -> psum_t[p, t, m, s] = x_m[s, 4p+t]
                        nc.tensor.transpose(psum_t[:, t], tin[:, q, :, t], ident[:])
                    # psum (t, m, s) -> tout[(2q+m), t, s]: dest viewed (part, t, m, s)
                    dest = tout[:, 2 * q:2 * q + 2, :, :].transpose([0, 2, 1, 3])
                    nc.vector.tensor_copy(out=dest, in_=psum_t[:])
                # store 2MB on the ACT HWDGE ring: dram (mat16, part, ts) ->
                # (part, mat16, ts); 1KB contiguous runs
                nc.scalar.dma_start(out[sup].transpose([1, 0, 2]), tout[:])

        if repeat == 1:
            body()
        else:
            with tc.For_i(0, repeat, 1):
                body()
    nc.compile()
    return nc


def _get_nc(repeat: int = 1):
    if repeat not in _CACHE:
        _CACHE[repeat] = _build(repeat)
    return _CACHE[repeat]


def run(x: np.ndarray, trace: bool = False, repeat: int = 1, **spmd_kwargs):
    """Run on 8 cores; returns (full output, BassKernelResults)."""
    nc = _get_nc(repeat)
    x = np.ascontiguousarray(x, dtype=np.float32)
    in_maps = [
        {"x": x[i * B_PER:(i + 1) * B_PER].reshape(PAIRS, 128, N)}
        for i in range(N_CORES)
    ]
    res = bass_utils.run_bass_kernel_spmd(
        nc, in_maps, core_ids=list(range(N_CORES)), trace=trace, **spmd_kwargs
    )
    outs = [r["out"].reshape(B_PER, C, N * S) for r in res.results]
    return np.concatenate(outs, axis=0), res


def kernel(x: np.ndarray) -> np.ndarray:
    out, _ = run(x)
    return out


# revision 44
# speedup vs baseline: 1.0210x; 1.0210x over previous
"""Trainium2 Bass kernel for nn_Catting_75058848465342.

Reference:  out = swapaxes(x[:, :, :64, :], -1, -2).reshape(B, C, N*S)
with x: [B=16, C=64, S=64, N=512] f32 — a pure data-movement problem
(transpose of the last two axes; the slice is the full dim).

Sharding: data-parallel over B across 8 NeuronCores (2 batches per core).

Per-core layout (memory-bound, target ~HBM roofline):
  * the 128 [S=64, N=512] matrices are processed in stacked PAIRS: the load
    DMA places a pair as an SBUF tile [128 part = (m, s), 512] so loads are
    2KB-contiguous per partition and 2MB per dma_start (8 pairs batched).
  * 4 TensorE transpose-mode matmuls per pair, with a stride-4 column access
    pattern on the stationary operand (columns n = 4p+t feed PSUM partition
    p), so PSUM partition p holds output columns n = 4p..4p+3 -> the store
    sees 1KB-contiguous HBM runs.
  * DVE copies PSUM->SBUF, reordering (t, m, s) -> (m, t, s).
  * load/store DMAs are 4MB per instruction (32 matrices); loads on the SP
    HWDGE ring with 4 buffers (the slower direction runs ahead), stores on
    the ACT ring with 2 buffers.

Measured on 8 axon trn2 cores: ~102.3 us/iteration per core for 32MB of HBM
traffic (~328 GB/s/core mixed R+W); DMA-bound — PE/DVE work is fully hidden
(a DMA-only variant measures the same time).  4MB DMAs with in4/out2
buffering edge out 2MB+sym-3 (~107.7) and 4MB+sym-3 (~103.5); 4MB+sym-2 is
worse (~112.6), as are half-size stores (~112).
"""
import sys

try:
    import concourse  # noqa: F401
except ImportError:
    sys.path.insert(0, "/opt/trn_rl_repo")

import numpy as np
from contextlib import ExitStack

from concourse import bacc, bass_utils, tile, masks
import concourse.mybir as mybir

F32 = mybir.dt.float32

N_CORES = 8
B, C, S, N = 16, 64, 64, 512
B_PER = B // N_CORES          # 2 batches per core
MATS = B_PER * C              # 128 [64,512] matrices per core
PAIRS = MATS // 2             # 64 stacked pairs
SUPER = 16                    # pairs per DMA super-tile (16 pairs = 32 mats = 4MB)
N_SUPER = PAIRS // SUPER      # 4 super-iterations
BUFS = 3

_CACHE = {}


def _build(repeat: int = 1, alt: bool = False, half2: bool = False, nsplit: int = 0,
           mode: str = "base", ld_gp: bool = False, st_gp: bool = False,
           sp: int = SUPER, bufs: int = BUFS, ibufs: int = 4, obufs: int = 2,
           half_store: bool = False):
    """nsplit: issue each load/store as nsplit equal dma_starts on its ring
    (0/1 = single instruction; half2 is legacy alias for nsplit=2).
    mode: base | wide2 (wide2: pair mats stacked in FREE dim -> 2KB store runs,
    half-partition loads on both rings, row-packed 64-row transposes).
    ld_gp/st_gp: carry half of each load/store on the SWDGE (gpsimd) path."""
    if half2:
        nsplit = 2
    nsplit = max(nsplit, 1)
    n_super = PAIRS // sp
    nc = bacc.Bacc("TRN2", target_bir_lowering=False, debug=False, num_devices=N_CORES)
    if mode == "wide2":
        return _build_wide2(nc, repeat)
    # x per core: [64 pairs, 128 rows=(m,s), 512 cols=n]  (same bytes as
    # [2, 64, 64, 512] row-major)
    x = nc.dram_tensor("x", [PAIRS, 128, N], F32, kind="ExternalInput").ap()
    # out per core: [sup, mat16, p, (t,s)] — flat bytes equal out[mat, n*64+s]
    out = nc.dram_tensor("out", [n_super, 2 * sp, 128, 256], F32,
                         kind="ExternalOutput").ap()

    with ExitStack() as ctx:
        tc = ctx.enter_context(tile.TileContext(nc))
        const_pool = ctx.enter_context(tc.tile_pool(name="const", bufs=1))
        in_pool = ctx.enter_context(tc.tile_pool(name="in", bufs=ibufs or bufs))
        out_pool = ctx.enter_context(tc.tile_pool(name="out", bufs=obufs or bufs))
        psum_pool = ctx.enter_context(tc.tile_pool(name="psum", bufs=8, space="PSUM"))

        ident = const_pool.tile([128, 128], F32)
        masks.make_identity(nc, ident[:])

        def body():
            for sup in range(n_super):
                ld = nc.sync if (not alt or sup % 2 == 0) else nc.scalar
                st = nc.scalar if (not alt or sup % 2 == 0) else nc.sync
                # load 8 pairs = 2MB: dram (pair', part, n) -> (part, pair', n)
                tin = in_pool.tile([128, sp, 128, 4], F32)  # (pair', n_hi, t)
                xs = x[sup * sp:(sup + 1) * sp]
                if ld_gp:
                    h = sp // 2
                    ld.dma_start(tin[:, :h], xs[:h].transpose([1, 0, 2]))
                    nc.gpsimd.dma_start(tin[:, h:], xs[h:].transpose([1, 0, 2]))
                else:
                    h = sp // nsplit
                    for k in range(nsplit):
                        ld.dma_start(tin[:, k * h:(k + 1) * h],
                                     xs[k * h:(k + 1) * h].transpose([1, 0, 2]))
                if half_store:
                    hp = sp // 2
                    for hf in range(2):
                        tout = out_pool.tile([128, 2 * hp, 4, 64], F32)
                        for q2 in range(hp):
                            q = hf * hp + q2
                            psum_t = psum_pool.tile([128, 4, 2, 64], F32)
                            for t in range(4):
                                nc.tensor.transpose(psum_t[:, t], tin[:, q, :, t],
                                                    ident[:])
                            dest = tout[:, 2 * q2:2 * q2 + 2, :, :].transpose(
                                [0, 2, 1, 3])
                            nc.vector.tensor_copy(out=dest, in_=psum_t[:])
                        st.dma_start(
                            out[sup, hf * 2 * hp:(hf + 1) * 2 * hp].transpose([1, 0, 2]),
                            tout[:])
                    continue
                tout = out_pool.tile([128, 2 * sp, 4, 64], F32)  # ((pair',m), t, s)
                for q in range(sp):
                    psum_t = psum_pool.tile([128, 4, 2, 64], F32)  # one bank: (t, m, s)
                    for t in range(4):
                        # stationary = tin[:, q, :, t]: [128 part, 128 cols stride 4]
                        # -> psum_t[p, t, m, s] = x_m[s, 4p+t]
                        nc.tensor.transpose(psum_t[:, t], tin[:, q, :, t], ident[:])
                    # psum (t, m, s) -> tout[(2q+m), t, s]: dest viewed (part, t, m, s)
                    dest = tout[:, 2 * q:2 * q + 2, :, :].transpose([0, 2, 1, 3])
                    nc.vector.tensor_copy(out=dest, in_=psum_t[:])
                # store 2MB on the ACT HWDGE ring: dram (mat16, part, ts) ->
                # (part, mat16, ts); 1KB contiguous runs
                if st_gp:
                    g = sp
                    st.dma_start(out[sup, :g].transpose([1, 0, 2]), tout[:, :g])
                    nc.gpsimd.dma_start(out[sup, g:].transpose([1, 0, 2]), tout[:, g:])
                else:
                    g = 2 * sp // nsplit
                    for k in range(nsplit):
                        st.dma_start(out[sup, k * g:(k + 1) * g].transpose([1, 0, 2]),
                                     tout[:, k * g:(k + 1) * g])

        if repeat == 1:
            body()
        else:
            with tc.For_i(0, repeat, 1):
                body()
    nc.compile()
    return nc


def _build_wide2(nc, repeat: int):
    """2KB-store-run layout.

    x viewed as [sup 8, half 2, q2 4, m 2, s 64, n 512]; per super-iteration
    two 1MB loads (halves on sync/scalar) fill tin[128, q2, m, n_hi, t8]:
    partitions 0-63 = s-rows of half-0 pairs, 64-127 = half-1 pairs.
    Transpose t of pair (half, q2): stationary = tin[half, q2, :, :, t]
    (128 cols stride 8 spanning both m) -> psum[p, t, s] with p<64 = mat m0
    col 8p+t, p>=64 = mat m1 col 8(p-64)+t.  All outputs at PSUM partition 0;
    A/B-half matmuls occupy different row groups -> concurrent on the array.
    Store: [128, 2KB] contiguous per pair, 2MB per instruction.
    """
    n_super = N_SUPER
    x = nc.dram_tensor("x", [n_super, 2, 4, 2, 64, N], F32, kind="ExternalInput").ap()
    out = nc.dram_tensor("out", [n_super, SUPER, 128, 512], F32,
                         kind="ExternalOutput").ap()

    with ExitStack() as ctx:
        tc = ctx.enter_context(tile.TileContext(nc))
        const_pool = ctx.enter_context(tc.tile_pool(name="const", bufs=1))
        in_pool = ctx.enter_context(tc.tile_pool(name="in", bufs=BUFS))
        out_pool = ctx.enter_context(tc.tile_pool(name="out", bufs=BUFS))
        psum_pool = ctx.enter_context(tc.tile_pool(name="psum", bufs=8, space="PSUM"))

        ident = const_pool.tile([128, 128], F32)
        masks.make_identity(nc, ident[:])
        # identity blocks on both partition halves: ident_b[64h+i, j] = d(i, j)
        ident_b = const_pool.tile([128, 64], F32)
        nc.gpsimd.memset(ident_b[:], 0.0)
        nc.vector.tensor_copy(out=ident_b[0:64, :], in_=ident[0:64, 0:64])
        nc.sync.dma_start(ident_b[64:128, :], ident[0:64, 0:64])  # partition shift

        def body():
            for sup in range(n_super):
                # free = (q2, m, n_hi, t8); partition = (half, s)
                tin = in_pool.tile([128, 4, 2, 64, 8], F32)
                # per half: dram (q2, m, s, n) -> (s, q2, m, n); (q2, m) merges
                nc.sync.dma_start(tin[0:64], x[sup, 0].transpose([2, 0, 1, 3]))
                nc.scalar.dma_start(tin[64:128], x[sup, 1].transpose([2, 0, 1, 3]))
                tout = out_pool.tile([128, SUPER, 8, 64], F32)  # (pair', t, s)
                for q2 in range(4):
                    ps_a = psum_pool.tile([128, 8, 64], F32, tag="ps")
                    ps_b = psum_pool.tile([128, 8, 64], F32, tag="ps")
                    for t in range(8):
                        # interleave halves: different row groups -> concurrent
                        nc.tensor.transpose(ps_a[:, t], tin[0:64, q2, :, :, t],
                                            ident_b[0:64, :])
                        nc.tensor.transpose(ps_b[:, t], tin[64:128, q2, :, :, t],
                                            ident_b[64:128, :])
                    nc.vector.tensor_copy(out=tout[:, q2], in_=ps_a[:])
                    nc.vector.tensor_copy(out=tout[:, 4 + q2], in_=ps_b[:])
                st = nc.scalar if sup % 2 == 0 else nc.sync
                st.dma_start(out[sup].transpose([1, 0, 2]), tout[:])

        if repeat == 1:
            body()
        else:
            with tc.For_i(0, repeat, 1):
                body()
    nc.compile()
    return nc


def _get_nc(repeat: int = 1, **kw):
    key = (repeat, tuple(sorted(kw.items())))
    if key not in _CACHE:
        _CACHE[key] = _build(repeat, **kw)
    return _CACHE[key]


def run(x: np.ndarray, trace: bool = False, repeat: int = 1,
        build_kw: dict | None = None, **spmd_kwargs):
    """Run on 8 cores; returns (full output, BassKernelResults)."""
    build_kw = build_kw or {}
    nc = _get_nc(repeat, **build_kw)
    x = np.ascontiguousarray(x, dtype=np.float32)
    if build_kw.get("mode") == "wide2":
        shp = (N_SUPER, 2, 4, 2, 64, N)
    else:
        shp = (PAIRS, 128, N)
    in_maps = [
        {"x": x[i * B_PER:(i + 1) * B_PER].reshape(shp)}
        for i in range(N_CORES)
    ]
    res = bass_utils.run_bass_kernel_spmd(
        nc, in_maps, core_ids=list(range(N_CORES)), trace=trace, **spmd_kwargs
    )
    outs = [r["out"].reshape(B_PER, C, N * S) for r in res.results]
    return np.concatenate(outs, axis=0), res


def kernel(x: np.ndarray) -> np.ndarray:
    out, _ = run(x)
    return out
